# revision 23
# baseline (speedup 1.0000x reference)
"""Trainium2 Bass kernel for nn_DecoderLSTM (Show-Attend-Tell style decoder).

Strategy (8 NeuronCores):
  Phase 1 (recurrence): batch-parallel, 8 samples/core, zero cross-core
    traffic. States are kept feature-major ([feat_part, batch_free]) so the
    per-step matmuls use cheap skinny streaming operands. Attention scores /
    context use a block-diagonal trick so each step is a few dense matmuls.
    Per-step hidden states (masked for prediction) accumulate in SBUF
    feature-major.
  AllGather: each core contributes its [512, 152] (bf16) hidden-state block;
    every core ends with all 1216 rows.
  Phase 2 (fc): vocab-parallel, each core computes all 1216 rows x its 3750
    vocab columns: preds = (mask*h_new) @ W_fc [+ mask x b_fc].

Numerics: matmuls in bf16 (weights+activations), all carries/accumulation in
fp32 (h/c state is fp32; bf16 only at matmul inputs).

Outputs caps/decode_len/sort_ind and the final unshard are host-side numpy.
"""

import math
import sys
from contextlib import ExitStack

import numpy as np

sys.path.insert(0, "/opt/trn_rl_repo")

import ml_dtypes  # noqa: E402

import concourse.bass as bass  # noqa: E402
import concourse.mybir as mybir  # noqa: E402
import concourse.tile as tile  # noqa: E402
from concourse.bass_utils import run_bass_kernel_spmd  # noqa: E402

B, P, ENC, DEC, ATT, EMB, V, T = 64, 196, 512, 512, 512, 512, 30000, 20
NS = T - 1            # 19 decode steps
NCORE = 8
BL = B // NCORE       # 8 samples per core
R = BL * P            # 1568 (b_local, p) rows per core
NT = (R + 127) // 128  # 13 partition tiles over R (last has 32 rows)
VS = V // NCORE       # 3750 vocab columns per core
RL = BL * NS          # 152 fc rows per core, r_local = t*8 + b_local
ROWS = B * NS         # 1216 fc rows total, r_glob = core*152 + r_local
NMT_FC = (ROWS + 127) // 128  # 10 (9x128 + 64)
SCALE = 1.0 / math.sqrt(ATT)

F32 = mybir.dt.float32
BF16 = mybir.dt.bfloat16
BF = ml_dtypes.bfloat16

LAST_RESULTS = {}  # stashed for test.py (exec time etc.)


def _fc_chunks():
    out = []
    c0 = 0
    while c0 < VS:
        out.append((c0, min(512, VS - c0)))
        c0 += 512
    return out


def _seg_b(mt):
    """Partition segments of tile mt over rows r=mt*128+p, grouped by b=r//196.

    Returns list of (p0, p1, b)."""
    segs = []
    r0, r1 = mt * 128, min(mt * 128 + 128, R)
    r = r0
    while r < r1:
        b = r // 196
        rend = min((b + 1) * 196, r1)
        segs.append((r - r0, rend - r0, b))
        r = rend
    return segs


def fix_multiwait(nc):
    """This container's walrus accepts at most ONE sync wait per instruction.
    Split extra waits onto preceding same-engine NOPs (engine program order
    makes this equivalent)."""
    for fn in nc.m.functions:
        for bl in fn.blocks:
            new = []
            for inst in bl.instructions:
                si = inst.sync_info
                if si is not None and si.on_wait and len(si.on_wait) > 1:
                    waits = list(si.on_wait)
                    for j, w in enumerate(waits[:-1]):
                        new.append(
                            mybir.InstNoOp(
                                name=f"{inst.name}-wsplit{j}",
                                engine=inst.engine,
                                sync_info=mybir.SyncInfo(
                                    on_wait=[w], on_update=[]
                                ),
                            )
                        )
                    inst.sync_info = mybir.SyncInfo(
                        on_wait=[waits[-1]], on_update=list(si.on_update)
                    )
                new.append(inst)
            bl.instructions = new
    return nc


def build_program(use_bfc, zbias, n_steps=NS, do_pre=True, do_fc=True,
                  do_alpha=True, do_ag=True):
    nc = bass.Bass(
        "TRN2", target_bir_lowering=False, debug=False, num_devices=NCORE
    )

    def inp(name, shape, dt):
        return nc.dram_tensor(name, list(shape), dt, kind="ExternalInput").ap()

    # ---- external inputs (per-core host-prepped layouts) ----
    eoT = inp("eoT", [512, R], BF16)          # [enc, (b,p)] feature-major
    embsT = inp("embsT", [512, RL], BF16)     # [emb, (t,b)]
    wq = inp("wq", [512, 512], BF16)          # [in, out] = lhsT layouts
    wk = inp("wk", [512, 512], BF16)
    wv = inp("wv", [512, 512], BF16)
    wbeta = inp("wbeta", [512, 512], BF16)
    wih0 = inp("wih0", [512, 512], BF16)      # W_init_h
    wic0 = inp("wic0", [512, 512], BF16)      # W_init_c
    wtop = inp("wtop", [512, 2048], BF16)     # W_ih[:512]  (emb part)
    wcat = inp("wcat", [1024, 2048], BF16)    # [W_ih[512:]; W_hh]
    wfc = inp("wfc", [512, VS], BF16)         # vocab slice
    # biases, feature-major [128, n_mtiles]
    bq = inp("bq", [128, 4], F32)
    bk = inp("bk", [128, 4], F32)
    bbeta = inp("bbeta", [128, 4], F32)
    bih0 = inp("bih0", [128, 4], F32)
    bic0 = inp("bic0", [128, 4], F32)
    bcat = inp("bcat", [128, 16], F32)        # b_ih + b_hh
    bv_row = inp("bv_row", [1, 512], BF16)    # att_v bias as K=1 matmul row
    ones_row = inp("ones_row", [1, 128], BF16)
    ones_blk = inp("ones_blk", [128, NT, BL], F32)   # blockdiag ones (b,p)-major
    onesT_blk = inp("onesT_blk", [8, NT, 128], F32)  # transposed blockdiag
    ones_col = inp("ones_col", [1, 128], F32)        # rank-1 bcast lhsT
    i8 = inp("i8", [8, 8], F32)                      # identity for PE transpose
    mask_sel = inp("mask_sel", [128, NS, 4, BL], F32)  # step mask, part-bcast
    mask_bd = inp("mask_bd", [128, NT, NS], F32)       # mask in (b,p)-major
    maskrow = inp("maskrow", [1, ROWS], BF16)          # fc bias mask row
    bfc_row = inp("bfc_row", [1, VS], BF16)

    # ---- external outputs ----
    preds_o = nc.dram_tensor("preds", [ROWS, VS], F32, kind="ExternalOutput").ap()
    alphas_o = nc.dram_tensor(
        "alphas_raw", [128, NT, NS], F32, kind="ExternalOutput"
    ).ap()

    # H rows for step t are final right after step t: overlap the gather with
    # the remaining recurrence; only the last small chunk is exposed.
    AG_CUTS = [0, 7, 12, 16, NS]

    chunks = _fc_chunks()

    with tile.TileContext(nc) as tc, ExitStack() as ctx:
        const = ctx.enter_context(tc.tile_pool(name="const", bufs=1))
        state = ctx.enter_context(tc.tile_pool(name="state", bufs=1))
        dramp = ctx.enter_context(tc.tile_pool(name="dramp", bufs=1, space="DRAM"))

        # chunked-AllGather bounce buffers (pool tiles so Tile tracks deps)
        hag_ins, hag_outs = [], []
        for j in range(len(AG_CUTS) - 1):
            w = (AG_CUTS[j + 1] - AG_CUTS[j]) * BL
            hag_ins.append(dramp.tile(
                [512, w], BF16, name=f"hag_in{j}", tag=f"hag_in{j}"))
            hag_outs.append(dramp.tile(
                [NCORE, 512, w], BF16, name=f"hag_out{j}",
                tag=f"hag_out{j}", addr_space="Shared"))

        def cload(ap_in, shape, dt, tag, rearr=None, **kw):
            t = const.tile(list(shape), dt, name=tag, tag=tag)
            src = ap_in if rearr is None else ap_in.rearrange(rearr, **kw)
            nc.sync.dma_start(out=t[:], in_=src)
            return t

        # SBUF-resident constants.  [512, N] mats go in as [128, 4, N].
        eoT_s = cload(eoT, [128, 4, R], BF16, "eoT_s", "(kh kl) r -> kl kh r", kl=128)
        embsT_s = cload(embsT, [128, 4, RL], BF16, "embsT_s", "(kh kl) r -> kl kh r", kl=128)
        wq_s = cload(wq, [128, 4, 512], BF16, "wq_s", "(kh kl) m -> kl kh m", kl=128)
        wk_s = cload(wk, [128, 4, 512], BF16, "wk_s", "(kh kl) m -> kl kh m", kl=128)
        wv_s = cload(wv, [128, 4, 512], BF16, "wv_s", "(kh kl) m -> kl kh m", kl=128)
        wbeta_s = cload(wbeta, [128, 4, 512], BF16, "wbeta_s", "(kh kl) m -> kl kh m", kl=128)
        wih0_s = cload(wih0, [128, 4, 512], BF16, "wih0_s", "(kh kl) m -> kl kh m", kl=128)
        wic0_s = cload(wic0, [128, 4, 512], BF16, "wic0_s", "(kh kl) m -> kl kh m", kl=128)
        wtop_s = cload(wtop, [128, 4, 2048], BF16, "wtop_s", "(kh kl) m -> kl kh m", kl=128)
        wcat_s = cload(wcat, [128, 8, 2048], BF16, "wcat_s", "(kh kl) m -> kl kh m", kl=128)
        wfc_s = cload(wfc, [128, 4, VS], BF16, "wfc_s", "(kh kl) m -> kl kh m", kl=128)
        bq_s = cload(bq, [128, 4], F32, "bq_s")
        bk_s = cload(bk, [128, 4], F32, "bk_s")
        bbeta_s = cload(bbeta, [128, 4], F32, "bbeta_s")
        bih0_s = cload(bih0, [128, 4], F32, "bih0_s")
        bic0_s = cload(bic0, [128, 4], F32, "bic0_s")
        bcat_s = cload(bcat, [128, 16], F32, "bcat_s")
        bv_row_s = cload(bv_row, [1, 512], BF16, "bv_row_s")
        ones_row_s = cload(ones_row, [1, 128], BF16, "ones_row_s")
        ones_blk_s = cload(ones_blk, [128, NT, BL], F32, "ones_blk_s")
        onesT_blk_s = cload(onesT_blk, [8, NT, 128], F32, "onesT_blk_s")
        ones_col_s = cload(ones_col, [1, 128], F32, "ones_col_s")
        i8_s = cload(i8, [8, 8], F32, "i8_s")
        mask_sel_s = cload(mask_sel, [128, NS, 4, BL], F32, "mask_sel_s")
        mask_bd_s = cload(mask_bd, [128, NT, NS], F32, "mask_bd_s")
        maskrow_s = cload(maskrow, [1, ROWS], BF16, "maskrow_s")
        bfc_row_s = cload(bfc_row, [1, VS], BF16, "bfc_row_s")

        # persistent working tensors
        def st(shape, dt, tag):
            return state.tile(list(shape), dt, name=tag, tag=tag)

        att_kT_s = st([128, 4, R], BF16, "att_kT_s")     # [att_feat, (b,p)]
        att_v_s = st([128, NT, 512], BF16, "att_v_s")    # [(b,p), enc]
        e_all_s = st([128, NT, NS], F32, "e_all_s")      # exp(relu(s)) per step
        r_all_s = st([8, NS], F32, "r_all_s")            # 1/denominator per step
        E_all_s = st([128, 16, NS, BL], F32, "E_all_s")  # embs @ W_ih[:512] + b
        H_cT_s = st([128, 4, NS, BL], BF16, "H_cT_s")    # masked h_new, fmajor
        h_f32 = st([128, 4, BL], F32, "h_f32")
        c_f32 = st([128, 4, BL], F32, "c_f32")
        hT_bf = st([128, 4, BL], BF16, "hT_bf")
        mean_f = st([128, 4, BL], F32, "mean_f")
        mean_bf = st([128, 4, BL], BF16, "mean_bf")
        qT_bf = st([128, 4, BL], BF16, "qT_bf")
        beta_sig = st([128, 4, BL], F32, "beta_sig")
        beta_r = st([128, 4, BL], F32, "beta_r")
        xi_att = st([128, 4, BL], BF16, "xi_att")
        s_bd = st([128, NT], F32, "s_bd")
        sc_scr = st([128, NT, BL], F32, "sc_scr")
        e_blk = st([128, NT, BL], BF16, "e_blk")
        gpre = st([128, 16, BL], F32, "gpre")
        gact = st([128, 16, BL], F32, "gact")
        tmp1 = st([128, 4, BL], F32, "tmp1")
        tmp2 = st([128, 4, BL], F32, "tmp2")
        c_new = st([128, 4, BL], F32, "c_new")
        th_c = st([128, 4, BL], F32, "th_c")
        h_new = st([128, 4, BL], F32, "h_new")
        rT_sb = st([1, 8], F32, "rT_sb")
        alpha_st = st([128, NT, NS], F32, "alpha_st")
        H_T_s = st([128, 4, ROWS], BF16, "H_T_s")

        # zero the pad region of s_bd once; extraction never writes pads.
        nc.vector.memset(s_bd[:], 0.0)

        # ================= phase A: precompute =================
        with ExitStack() as pa:
            pc = pa.enter_context(tc.tile_pool(name="pc_ps", bufs=3, space="PSUM"))
            mm8 = pa.enter_context(tc.tile_pool(name="mm8a", bufs=2, space="PSUM"))

            # meanT over p: eoT_s viewed [128, 4, BL, P] reduce innermost
            nc.vector.tensor_reduce(
                out=mean_f[:],
                in_=eoT_s[:].rearrange("k e (b p) -> k e b p", p=P),
                axis=mybir.AxisListType.X,
                op=mybir.AluOpType.add,
            )
            nc.scalar.activation(
                out=mean_bf[:], in_=mean_f[:],
                func=mybir.ActivationFunctionType.Copy, scale=1.0 / P,
            )

            # h0 / c0  (feature-major [dec, b])
            for dst, w_s, b_s, zb in (
                (h_f32, wih0_s, bih0_s, zbias["b_init_h"]),
                (c_f32, wic0_s, bic0_s, zbias["b_init_c"]),
            ):
                ps = mm8.tile([128, 4, BL], F32, name="ps_h0", tag="mm8a")
                for mt in range(4):
                    for kt in range(4):
                        nc.tensor.matmul(
                            ps[:, mt, :],
                            w_s[:, kt, mt * 128:(mt + 1) * 128],
                            mean_bf[:, kt, :],
                            start=(kt == 0), stop=(kt == 3),
                        )
                if zb:
                    nc.scalar.activation(
                        out=dst[:], in_=ps[:],
                        func=mybir.ActivationFunctionType.Identity,
                    )
                else:
                    for mt in range(4):
                        nc.scalar.activation(
                            out=dst[:, mt, :], in_=ps[:, mt, :],
                            func=mybir.ActivationFunctionType.Identity,
                            bias=b_s[:, mt:mt + 1],
                        )
            nc.vector.tensor_copy(hT_bf[:], h_f32[:])

            # att_kT = (eo @ Wk + bk)^T : out [att_feat 4mt, (b,p)]
            cch = [(0, 512), (512, 512), (1024, 512), (1536, R - 1536)]
            for mt in range(4):
                for c0, csz in cch:
                    ps = pc.tile([128, 512], F32, name="ps_pc", tag="pc")
                    for kt in range(4):
                        nc.tensor.matmul(
                            ps[:, :csz],
                            wk_s[:, kt, mt * 128:(mt + 1) * 128],
                            eoT_s[:, kt, c0:c0 + csz],
                            start=(kt == 0), stop=(kt == 3),
                        )
                    if zbias["bk"]:
                        nc.scalar.activation(
                            out=att_kT_s[:, mt, c0:c0 + csz], in_=ps[:, :csz],
                            func=mybir.ActivationFunctionType.Identity,
                        )
                    else:
                        nc.scalar.activation(
                            out=att_kT_s[:, mt, c0:c0 + csz], in_=ps[:, :csz],
                            func=mybir.ActivationFunctionType.Identity,
                            bias=bk_s[:, mt:mt + 1],
                        )

            # att_v (b,p)-major: out [(b,p) 13mt, enc 512]
            for mt in range(NT):
                msz = min(128, R - mt * 128)
                ps = pc.tile([128, 512], F32, name="ps_pv", tag="pc")
                for kt in range(4):
                    nc.tensor.matmul(
                        ps[:msz, :],
                        eoT_s[:, kt, mt * 128:mt * 128 + msz],
                        wv_s[:, kt, :],
                        start=(kt == 0),
                        stop=(kt == 3 and zbias["bv"]),
                    )
                if not zbias["bv"]:
                    # + bv (rank-1; bv broadcast over rows)
                    nc.tensor.matmul(
                        ps[:msz, :],
                        ones_row_s[0:1, :msz],
                        bv_row_s[0:1, :],
                        start=False, stop=True,
                    )
                nc.vector.tensor_copy(att_v_s[:msz, mt, :], ps[:msz, :])

            # E_all = embs @ W_ih[:512] + (b_ih + b_hh)  (gate-feature-major)
            for mt in range(16):
                ps = pc.tile([128, RL], F32, name="ps_pe", tag="pc")
                for kt in range(4):
                    nc.tensor.matmul(
                        ps[:],
                        wtop_s[:, kt, mt * 128:(mt + 1) * 128],
                        embsT_s[:, kt, :],
                        start=(kt == 0), stop=(kt == 3),
                    )
                if zbias["b_ih_hh"]:
                    nc.scalar.activation(
                        out=E_all_s[:, mt, :, :].rearrange("p t b -> p (t b)"),
                        in_=ps[:],
                        func=mybir.ActivationFunctionType.Identity,
                    )
                else:
                    nc.scalar.activation(
                        out=E_all_s[:, mt, :, :].rearrange("p t b -> p (t b)"),
                        in_=ps[:],
                        func=mybir.ActivationFunctionType.Identity,
                        bias=bcat_s[:, mt:mt + 1],
                    )

        # ================= phase B: recurrence =================
        with ExitStack() as pb:
            mm8 = pb.enter_context(tc.tile_pool(name="mm8", bufs=1, space="PSUM"))
            scp = pb.enter_context(tc.tile_pool(name="scp", bufs=1, space="PSUM"))
            tny = pb.enter_context(tc.tile_pool(name="tny", bufs=2, space="PSUM"))

            for t in range(n_steps):
                mk = mask_sel_s[:, t, :, :]
                # ---- q = h @ Wq + bq   (feature-major out) ----
                q_ps = mm8.tile([128, 4, BL], F32, name="q_ps", tag="q_ps")
                for mt in range(4):
                    for kt in range(4):
                        nc.tensor.matmul(
                            q_ps[:, mt, :],
                            wq_s[:, kt, mt * 128:(mt + 1) * 128],
                            hT_bf[:, kt, :],
                            start=(kt == 0), stop=(kt == 3),
                        )
                if zbias["bq"]:
                    nc.scalar.activation(
                        out=qT_bf[:], in_=q_ps[:],
                        func=mybir.ActivationFunctionType.Identity,
                    )
                else:
                    for mt in range(4):
                        nc.scalar.activation(
                            out=qT_bf[:, mt, :], in_=q_ps[:, mt, :],
                            func=mybir.ActivationFunctionType.Identity,
                            bias=bq_s[:, mt:mt + 1],
                        )
                # ---- beta = sigmoid(h @ Wbeta + b) ----
                b_ps = mm8.tile([128, 4, BL], F32, name="b_ps", tag="b_ps")
                for mt in range(4):
                    for kt in range(4):
                        nc.tensor.matmul(
                            b_ps[:, mt, :],
                            wbeta_s[:, kt, mt * 128:(mt + 1) * 128],
                            hT_bf[:, kt, :],
                            start=(kt == 0), stop=(kt == 3),
                        )
                if zbias["b_beta"]:
                    nc.scalar.activation(
                        out=beta_sig[:], in_=b_ps[:],
                        func=mybir.ActivationFunctionType.Sigmoid,
                    )
                else:
                    for mt in range(4):
                        nc.scalar.activation(
                            out=beta_sig[:, mt, :], in_=b_ps[:, mt, :],
                            func=mybir.ActivationFunctionType.Sigmoid,
                            bias=bbeta_s[:, mt:mt + 1],
                        )

                # ---- scores (cross vs own q) into ONE psum tile ----
                sc_ps = scp.tile([128, NT, BL], F32, name="sc_ps", tag="sc_ps")
                for mt in range(NT):
                    msz = min(128, R - mt * 128)
                    for kt in range(4):
                        nc.tensor.matmul(
                            sc_ps[:msz, mt, :],
                            att_kT_s[:, kt, mt * 128:mt * 128 + msz],
                            qT_bf[:, kt, :],
                            start=(kt == 0), stop=(kt == 3),
                        )
                # diagonal extraction in two full-tile DVE ops; pad rows of
                # the last mtile hit zero blockdiag entries so they vanish.
                nc.vector.tensor_mul(sc_scr[:], sc_ps[:], ones_blk_s[:])
                nc.vector.tensor_reduce(
                    out=s_bd[:],
                    in_=sc_scr[:],
                    axis=mybir.AxisListType.X,
                    op=mybir.AluOpType.add,
                )

                # ---- exp(relu(s) * scale) ----
                nc.vector.tensor_scalar_max(s_bd[:], s_bd[:], 0.0)
                nc.scalar.activation(
                    out=e_all_s[:, :, t], in_=s_bd[:],
                    func=mybir.ActivationFunctionType.Exp, scale=SCALE,
                )

                # ---- denominators, reciprocal, partition-broadcast ----
                dn_ps = tny.tile([8, 1], F32, name="dn_ps", tag="tny")
                for kt in range(NT):
                    nc.tensor.matmul(
                        dn_ps[:],
                        ones_blk_s[:, kt, :],
                        e_all_s[:, kt, t:t + 1],
                        start=(kt == 0), stop=(kt == NT - 1),
                    )
                nc.vector.reciprocal(r_all_s[:, t:t + 1], dn_ps[:])
                rt_ps = tny.tile([1, 8], F32, name="rt_ps", tag="tny")
                nc.tensor.transpose(rt_ps[:], r_all_s[:, t:t + 1], i8_s[:])
                nc.scalar.copy(rT_sb[:], rt_ps[:])
                rbc_ps = tny.tile([128, 8], F32, name="rbc_ps", tag="tny")
                nc.tensor.matmul(
                    rbc_ps[:], ones_col_s[0:1, :], rT_sb[0:1, :],
                    start=True, stop=True,
                )
                for mt in range(4):
                    nc.vector.tensor_mul(
                        beta_r[:, mt, :], beta_sig[:, mt, :], rbc_ps[:]
                    )

                # ---- blockdiag unnormalized alpha ----
                for b in range(BL):
                    nc.vector.tensor_mul(
                        e_blk[:, :, b], ones_blk_s[:, :, b], e_all_s[:, :, t]
                    )

                # ---- att context (unnormalized), then * beta * (1/denom) ----
                a_ps = mm8.tile([128, 4, BL], F32, name="a_ps", tag="a_ps")
                for mt in range(4):
                    for kt in range(NT):
                        ksz = min(128, R - kt * 128)
                        nc.tensor.matmul(
                            a_ps[:, mt, :],
                            att_v_s[:ksz, kt, mt * 128:(mt + 1) * 128],
                            e_blk[:ksz, kt, :],
                            start=(kt == 0), stop=(kt == NT - 1),
                        )
                nc.vector.tensor_mul(xi_att[:], a_ps[:], beta_r[:])

                # ---- gates (gate-feature-major) into ONE psum tile ----
                g_ps = mm8.tile([128, 16, BL], F32, name="g_ps", tag="g_ps")
                for mt in range(16):
                    for kt in range(8):
                        rhs = xi_att[:, kt, :] if kt < 4 else hT_bf[:, kt - 4, :]
                        nc.tensor.matmul(
                            g_ps[:, mt, :],
                            wcat_s[:, kt, mt * 128:(mt + 1) * 128],
                            rhs,
                            start=(kt == 0), stop=(kt == 7),
                        )
                nc.vector.tensor_add(gpre[:], g_ps[:], E_all_s[:, :, t, :])
                nc.scalar.activation(
                    out=gact[:, 0:8, :], in_=gpre[:, 0:8, :],
                    func=mybir.ActivationFunctionType.Sigmoid,
                )
                nc.scalar.activation(
                    out=gact[:, 8:12, :], in_=gpre[:, 8:12, :],
                    func=mybir.ActivationFunctionType.Tanh,
                )
                nc.scalar.activation(
                    out=gact[:, 12:16, :], in_=gpre[:, 12:16, :],
                    func=mybir.ActivationFunctionType.Sigmoid,
                )

                # ---- LSTM cell (fp32 carries) ----
                nc.vector.tensor_mul(tmp1[:], gact[:, 4:8, :], c_f32[:])
                nc.vector.tensor_mul(tmp2[:], gact[:, 0:4, :], gact[:, 8:12, :])
                nc.vector.tensor_add(c_new[:], tmp1[:], tmp2[:])
                nc.scalar.activation(
                    out=th_c[:], in_=c_new[:],
                    func=mybir.ActivationFunctionType.Tanh,
                )
                nc.vector.tensor_mul(h_new[:], gact[:, 12:16, :], th_c[:])
                # masked prediction h -> H (bf16), masked state carry
                # (arithmetic blend: x += m * (x_new - x); mask is {0,1} f32)
                nc.vector.tensor_mul(H_cT_s[:, :, t, :], h_new[:], mk)
                nc.vector.tensor_sub(tmp1[:], h_new[:], h_f32[:])
                nc.vector.tensor_mul(tmp1[:], tmp1[:], mk)
                nc.vector.tensor_add(h_f32[:], h_f32[:], tmp1[:])
                nc.vector.tensor_sub(tmp2[:], c_new[:], c_f32[:])
                nc.vector.tensor_mul(tmp2[:], tmp2[:], mk)
                nc.vector.tensor_add(c_f32[:], c_f32[:], tmp2[:])
                nc.vector.tensor_copy(hT_bf[:], h_f32[:])

                if do_ag and (t + 1) in AG_CUTS:
                    j = AG_CUTS.index(t + 1) - 1
                    t0, t1 = AG_CUTS[j], AG_CUTS[j + 1]
                    nc.sync.dma_start(
                        out=hag_ins[j][:].rearrange(
                            "(eh el) (t b) -> el eh t b", el=128, t=t1 - t0
                        ),
                        in_=H_cT_s[:, :, t0:t1, :],
                    )
                    nc.gpsimd.collective_compute(
                        "AllGather",
                        mybir.AluOpType.bypass,
                        replica_groups=[list(range(NCORE))],
                        ins=[hag_ins[j][:].opt()],
                        outs=[hag_outs[j][:].opt()],
                    )
                    hro = hag_outs[j][:].rearrange(
                        "c (eh el) r -> c el eh r", el=128
                    )
                    for c in range(NCORE):
                        nc.sync.dma_start(
                            out=H_T_s[:, :,
                                      c * RL + t0 * BL:c * RL + t1 * BL],
                            in_=hro[c],
                        )

        # ================= alphas output =================
        with ExitStack() as pcx:
            r13 = pcx.enter_context(tc.tile_pool(name="r13", bufs=2, space="PSUM"))
            for mt in range(NT if do_alpha else 0):
                ps = r13.tile([128, NS], F32, name="ps_r13", tag="r13")
                nc.tensor.matmul(
                    ps[:], onesT_blk_s[:, mt, :], r_all_s[:], start=True, stop=True
                )
                nc.vector.tensor_mul(alpha_st[:, mt, :], e_all_s[:, mt, :], ps[:])
                nc.vector.tensor_mul(
                    alpha_st[:, mt, :], alpha_st[:, mt, :], mask_bd_s[:, mt, :]
                )
            if do_alpha:
                nc.sync.dma_start(out=alphas_o, in_=alpha_st[:])

        # (AllGather handled incrementally inside the step loop)

        # ================= phase C: fc =================
        with ExitStack() as pf:
            fcp = pf.enter_context(tc.tile_pool(name="fc_ps", bufs=6, space="PSUM"))
            stg = pf.enter_context(tc.tile_pool(name="fc_stage", bufs=4))
            for mt in range(NMT_FC if do_fc else 0):
                msz = min(128, ROWS - mt * 128)
                for (c0, csz) in chunks:
                    ps = fcp.tile([128, 512], F32, name="ps_fc", tag="fc")
                    for kt in range(4):
                        nc.tensor.matmul(
                            ps[:msz, :csz],
                            H_T_s[:, kt, mt * 128:mt * 128 + msz],
                            wfc_s[:, kt, c0:c0 + csz],
                            start=(kt == 0),
                            stop=(kt == 3 and not use_bfc),
                        )
                    if use_bfc:
                        nc.tensor.matmul(
                            ps[:msz, :csz],
                            maskrow_s[0:1, mt * 128:mt * 128 + msz],
                            bfc_row_s[0:1, c0:c0 + csz],
                            start=False, stop=True,
                        )
                    so = stg.tile([128, 512], F32, name="so_fc", tag="fcs")
                    if (mt + c0 // 512) % 2 == 0:
                        nc.vector.tensor_copy(so[:msz, :csz], ps[:msz, :csz])
                    else:
                        nc.scalar.copy(so[:msz, :csz], ps[:msz, :csz])
                    nc.sync.dma_start(
                        out=preds_o[mt * 128:mt * 128 + msz, c0:c0 + csz],
                        in_=so[:msz, :csz],
                    )

    fix_multiwait(nc)
    return nc


def _prep_inputs(inputs):
    """Host-side: sort, gather, transpose, cast. Returns (in_maps, host_outs)."""
    eo = np.asarray(inputs["encoder_out"], np.float32)
    caps_in = np.asarray(inputs["encoded_captions"])
    lens_in = np.asarray(inputs["caption_lens"])
    lens = lens_in[:, 0]
    sort_ind = np.argsort(-lens, kind="stable")
    lens_s = lens[sort_ind]
    eo_s = eo[sort_ind]                     # [B, P, ENC]
    caps_s = caps_in[sort_ind]              # [B, T]
    decode_len = lens_s - 1
    emb = np.asarray(inputs["emb"], np.float32)
    embs = emb[np.asarray(caps_s[:, :NS], np.int64)]  # [B, NS, EMB]

    w_ih = np.asarray(inputs["W_ih"], np.float32)
    w_hh = np.asarray(inputs["W_hh"], np.float32)
    b_fc = np.asarray(inputs["b_fc"], np.float32)
    use_bfc = bool(np.any(b_fc != 0.0))

    def fm(b):  # feature-major bias [128, nmt]
        b = np.asarray(b, np.float32)
        return np.ascontiguousarray(b.reshape(-1, 128).T)

    # blockdiag ones
    blk = np.zeros((NT * 128, BL), np.float32)
    rr = np.arange(R)
    blk[rr, rr // P] = 1.0
    ones_blk = np.ascontiguousarray(blk.reshape(NT, 128, BL).transpose(1, 0, 2))
    onesT_blk = np.ascontiguousarray(blk.reshape(NT, 128, BL).transpose(2, 0, 1))

    # masks: m[t, b] = t < decode_len[b]
    m_tb = (np.arange(NS)[:, None] < decode_len[None, :]).astype(np.float32)

    shared = dict(
        wq=np.asarray(inputs["Wq"], np.float32).astype(BF),
        wk=np.asarray(inputs["Wk"], np.float32).astype(BF),
        wv=np.asarray(inputs["Wv"], np.float32).astype(BF),
        wbeta=np.asarray(inputs["W_beta"], np.float32).astype(BF),
        wih0=np.asarray(inputs["W_init_h"], np.float32).astype(BF),
        wic0=np.asarray(inputs["W_init_c"], np.float32).astype(BF),
        wtop=np.ascontiguousarray(w_ih[:EMB]).astype(BF),
        wcat=np.ascontiguousarray(
            np.concatenate([w_ih[EMB:], w_hh], axis=0)
        ).astype(BF),
        bq=fm(inputs["bq"]),
        bk=fm(inputs["bk"]),
        bbeta=fm(inputs["b_beta"]),
        bih0=fm(inputs["b_init_h"]),
        bic0=fm(inputs["b_init_c"]),
        bcat=fm(np.asarray(inputs["b_ih"], np.float32)
                + np.asarray(inputs["b_hh"], np.float32)),
        bv_row=np.asarray(inputs["bv"], np.float32).reshape(1, ENC).astype(BF),
        ones_row=np.ones((1, 128), BF),
        ones_blk=ones_blk,
        onesT_blk=onesT_blk,
        ones_col=np.ones((1, 128), np.float32),
        i8=np.eye(8, dtype=np.float32),
    )

    # fc mask row over r_glob = c*152 + t*8 + b_local
    mrow = np.zeros((NCORE, NS, BL), np.float32)
    for c in range(NCORE):
        mrow[c] = m_tb[:, c * BL:(c + 1) * BL]
    shared["maskrow"] = mrow.reshape(1, ROWS).astype(BF)

    in_maps = []
    for c in range(NCORE):
        bsl = slice(c * BL, (c + 1) * BL)
        eoT = np.ascontiguousarray(
            eo_s[bsl].transpose(2, 0, 1).reshape(ENC, R)
        ).astype(BF)
        embsT = np.ascontiguousarray(
            embs[bsl].transpose(2, 1, 0).reshape(EMB, RL)
        ).astype(BF)
        m_c = m_tb[:, bsl]                             # [NS, BL]
        mask_sel = np.ascontiguousarray(
            np.broadcast_to(m_c[None, :, None, :], (128, NS, 4, BL))
        )
        mask_bd = np.zeros((NT * 128, NS), np.float32)
        mask_bd[rr] = m_c.T[rr // P]                   # [R rows, NS]
        mask_bd = np.ascontiguousarray(
            mask_bd.reshape(NT, 128, NS).transpose(1, 0, 2)
        )
        im = dict(shared)
        im.update(
            eoT=eoT,
            embsT=embsT,
            wfc=np.ascontiguousarray(
                np.asarray(inputs["W_fc"], np.float32)[:, c * VS:(c + 1) * VS]
            ).astype(BF),
            bfc_row=np.ascontiguousarray(b_fc[c * VS:(c + 1) * VS]).reshape(1, VS).astype(BF),
            mask_sel=mask_sel,
            mask_bd=mask_bd,
        )
        in_maps.append(im)

    def _z(name):
        return not bool(np.any(np.asarray(inputs[name], np.float32) != 0.0))

    zbias = dict(
        bq=_z("bq"), bk=_z("bk"), bv=_z("bv"), b_beta=_z("b_beta"),
        b_init_h=_z("b_init_h"), b_init_c=_z("b_init_c"),
        b_ih_hh=_z("b_ih") and _z("b_hh"),
    )
    host = dict(
        # match what the jax reference returns under default (x64-off) config
        caps=np.asarray(caps_s, np.int32),
        decode_len=np.asarray(decode_len, np.int32),
        sort_ind=sort_ind.astype(np.int32),
        use_bfc=use_bfc, zbias=zbias,
    )
    return in_maps, host


def kernel(**inputs):
    in_maps, host = _prep_inputs(inputs)
    nc = build_program(host["use_bfc"], host["zbias"])
    res = run_bass_kernel_spmd(
        nc, in_maps, core_ids=list(range(NCORE)), trace=False
    )
    LAST_RESULTS["res"] = res
    LAST_RESULTS["nc"] = nc

    predictions = np.empty((B, NS, V), np.float32)
    alphas = np.empty((B, NS, P), np.float32)
    for c in range(NCORE):
        out_c = res.results[c]
        pr = out_c["preds"].reshape(NCORE, NS, BL, VS)  # [c_batch, t, b_l, v]
        predictions[:, :, c * VS:(c + 1) * VS] = (
            pr.transpose(0, 2, 1, 3).reshape(B, NS, VS)
        )
        ar = out_c["alphas_raw"]                        # [128, NT, NS]
        flat = ar.transpose(1, 0, 2).reshape(NT * 128, NS)[:R]  # [(b,p), NS]
        alphas[c * BL:(c + 1) * BL] = (
            flat.reshape(BL, P, NS).transpose(0, 2, 1)
        )

    return (
        predictions,
        alphas,
        host["caps"],
        host["decode_len"],
        host["sort_ind"],
    )


if __name__ == "__main__":
    # smoke test with random inputs of the right shapes
    rng = np.random.default_rng(0)
    demo = dict(
        encoder_out=rng.standard_normal((B, P, ENC), np.float32),
        encoded_captions=rng.integers(0, V, (B, T)),
        caption_lens=rng.integers(2, T + 1, (B, 1)),
    )
    print("host prep only (no device):")
    in_maps, host = _prep_inputs(
        dict(
            demo,
            Wk=rng.standard_normal((ENC, ATT), np.float32) * 0.02,
            bk=np.zeros(ATT, np.float32),
            Wq=rng.standard_normal((DEC, ATT), np.float32) * 0.02,
            bq=np.zeros(ATT, np.float32),
            Wv=rng.standard_normal((ENC, ENC), np.float32) * 0.02,
            bv=np.zeros(ENC, np.float32),
            emb=rng.uniform(-0.1, 0.1, (V, EMB)).astype(np.float32),
            W_ih=rng.standard_normal((EMB + ENC, 4 * DEC), np.float32) * 0.02,
            b_ih=np.zeros(4 * DEC, np.float32),
            W_hh=rng.standard_normal((DEC, 4 * DEC), np.float32) * 0.02,
            b_hh=np.zeros(4 * DEC, np.float32),
            W_init_h=rng.standard_normal((ENC, DEC), np.float32) * 0.02,
            b_init_h=np.zeros(DEC, np.float32),
            W_init_c=rng.standard_normal((ENC, DEC), np.float32) * 0.02,
            b_init_c=np.zeros(DEC, np.float32),
            W_beta=rng.standard_normal((DEC, ENC), np.float32) * 0.02,
            b_beta=np.zeros(ENC, np.float32),
            W_fc=rng.standard_normal((DEC, V), np.float32) * 0.02,
            b_fc=np.zeros(V, np.float32),
        )
    )
    print("in_maps ready;", len(in_maps), "cores")
    nc = build_program(host["use_bfc"], host["zbias"])
    print("program built ok")


# revision 24
# speedup vs baseline: 1.0312x; 1.0312x over previous
"""Trainium2 Bass kernel for nn_DecoderLSTM (Show-Attend-Tell style decoder).

Strategy (8 NeuronCores):
  Phase 1 (recurrence): batch-parallel, 8 samples/core, zero cross-core
    traffic. States are kept feature-major ([feat_part, batch_free]) so the
    per-step matmuls use cheap skinny streaming operands. Attention scores /
    context use a block-diagonal trick so each step is a few dense matmuls.
    Per-step hidden states (masked for prediction) accumulate in SBUF
    feature-major.
  AllGather: each core contributes its [512, 152] (bf16) hidden-state block;
    every core ends with all 1216 rows.
  Phase 2 (fc): vocab-parallel, each core computes all 1216 rows x its 3750
    vocab columns: preds = (mask*h_new) @ W_fc [+ mask x b_fc].

Numerics: matmuls in bf16 (weights+activations), all carries/accumulation in
fp32 (h/c state is fp32; bf16 only at matmul inputs).

Outputs caps/decode_len/sort_ind and the final unshard are host-side numpy.
"""

import math
import sys
from contextlib import ExitStack

import numpy as np

sys.path.insert(0, "/opt/trn_rl_repo")

import ml_dtypes  # noqa: E402

import concourse.bass as bass  # noqa: E402
import concourse.mybir as mybir  # noqa: E402
import concourse.tile as tile  # noqa: E402
from concourse.bass_utils import run_bass_kernel_spmd  # noqa: E402

B, P, ENC, DEC, ATT, EMB, V, T = 64, 196, 512, 512, 512, 512, 30000, 20
NS = T - 1            # 19 decode steps
NCORE = 8
BL = B // NCORE       # 8 samples per core
R = BL * P            # 1568 (b_local, p) rows per core
NT = (R + 127) // 128  # 13 partition tiles over R (last has 32 rows)
VS = V // NCORE       # 3750 vocab columns per core
RL = BL * NS          # 152 fc rows per core, r_local = t*8 + b_local
ROWS = B * NS         # 1216 fc rows total, r_glob = core*152 + r_local
NMT_FC = (ROWS + 127) // 128  # 10 (9x128 + 64)
SCALE = 1.0 / math.sqrt(ATT)

F32 = mybir.dt.float32
BF16 = mybir.dt.bfloat16
BF = ml_dtypes.bfloat16

LAST_RESULTS = {}  # stashed for test.py (exec time etc.)


def _fc_chunks():
    out = []
    c0 = 0
    while c0 < VS:
        out.append((c0, min(512, VS - c0)))
        c0 += 512
    return out


def _seg_b(mt):
    """Partition segments of tile mt over rows r=mt*128+p, grouped by b=r//196.

    Returns list of (p0, p1, b)."""
    segs = []
    r0, r1 = mt * 128, min(mt * 128 + 128, R)
    r = r0
    while r < r1:
        b = r // 196
        rend = min((b + 1) * 196, r1)
        segs.append((r - r0, rend - r0, b))
        r = rend
    return segs


def fix_multiwait(nc):
    """This container's walrus accepts at most ONE sync wait per instruction.
    Split extra waits onto preceding same-engine NOPs (engine program order
    makes this equivalent)."""
    for fn in nc.m.functions:
        for bl in fn.blocks:
            new = []
            for inst in bl.instructions:
                si = inst.sync_info
                if si is not None and si.on_wait and len(si.on_wait) > 1:
                    waits = list(si.on_wait)
                    for j, w in enumerate(waits[:-1]):
                        new.append(
                            mybir.InstNoOp(
                                name=f"{inst.name}-wsplit{j}",
                                engine=inst.engine,
                                sync_info=mybir.SyncInfo(
                                    on_wait=[w], on_update=[]
                                ),
                            )
                        )
                    inst.sync_info = mybir.SyncInfo(
                        on_wait=[waits[-1]], on_update=list(si.on_update)
                    )
                new.append(inst)
            bl.instructions = new
    return nc


def build_program(use_bfc, zbias, n_steps=NS, do_pre=True, do_fc=True,
                  do_alpha=True, do_ag=True):
    nc = bass.Bass(
        "TRN2", target_bir_lowering=False, debug=False, num_devices=NCORE
    )

    def inp(name, shape, dt):
        return nc.dram_tensor(name, list(shape), dt, kind="ExternalInput").ap()

    # ---- external inputs (per-core host-prepped layouts) ----
    eoT = inp("eoT", [512, R], BF16)          # [enc, (b,p)] feature-major
    embsT = inp("embsT", [512, RL], BF16)     # [emb, (t,b)]
    wq = inp("wq", [512, 512], BF16)          # [in, out] = lhsT layouts
    wk = inp("wk", [512, 512], BF16)
    wv = inp("wv", [512, 512], BF16)
    wbeta = inp("wbeta", [512, 512], BF16)
    wih0 = inp("wih0", [512, 512], BF16)      # W_init_h
    wic0 = inp("wic0", [512, 512], BF16)      # W_init_c
    wtop = inp("wtop", [512, 2048], BF16)     # W_ih[:512]  (emb part)
    wcat = inp("wcat", [1024, 2048], BF16)    # [W_ih[512:]; W_hh]
    wfc = inp("wfc", [512, VS], BF16)         # vocab slice
    # biases, feature-major [128, n_mtiles]
    bq = inp("bq", [128, 4], F32)
    bk = inp("bk", [128, 4], F32)
    bbeta = inp("bbeta", [128, 4], F32)
    bih0 = inp("bih0", [128, 4], F32)
    bic0 = inp("bic0", [128, 4], F32)
    bcat = inp("bcat", [128, 16], F32)        # b_ih + b_hh
    bv_row = inp("bv_row", [1, 512], BF16)    # att_v bias as K=1 matmul row
    ones_row = inp("ones_row", [1, 128], BF16)
    ones_blk = inp("ones_blk", [128, NT, BL], F32)   # blockdiag ones (b,p)-major
    onesT_blk = inp("onesT_blk", [8, NT, 128], F32)  # transposed blockdiag
    ones_col = inp("ones_col", [1, 128], F32)        # rank-1 bcast lhsT
    i8 = inp("i8", [8, 8], F32)                      # identity for PE transpose
    mask_sel = inp("mask_sel", [128, NS, 4, BL], F32)  # step mask, part-bcast
    mask_bd = inp("mask_bd", [128, NT, NS], F32)       # mask in (b,p)-major
    maskrow = inp("maskrow", [1, ROWS], BF16)          # fc bias mask row
    bfc_row = inp("bfc_row", [1, VS], BF16)

    # ---- external outputs ----
    preds_o = nc.dram_tensor("preds", [ROWS, VS], F32, kind="ExternalOutput").ap()
    alphas_o = nc.dram_tensor(
        "alphas_raw", [128, NT, NS], F32, kind="ExternalOutput"
    ).ap()

    # H rows for step t are final right after step t: overlap the gather with
    # the remaining recurrence; only the last small chunk is exposed.
    AG_CUTS = [0, 7, 12, 16, NS]

    chunks = _fc_chunks()

    with tile.TileContext(nc) as tc, ExitStack() as ctx:
        const = ctx.enter_context(tc.tile_pool(name="const", bufs=1))
        state = ctx.enter_context(tc.tile_pool(name="state", bufs=1))
        dramp = ctx.enter_context(tc.tile_pool(name="dramp", bufs=1, space="DRAM"))

        # chunked-AllGather bounce buffers (pool tiles so Tile tracks deps)
        hag_ins, hag_outs = [], []
        for j in range(len(AG_CUTS) - 1):
            w = (AG_CUTS[j + 1] - AG_CUTS[j]) * BL
            hag_ins.append(dramp.tile(
                [512, w], BF16, name=f"hag_in{j}", tag=f"hag_in{j}"))
            hag_outs.append(dramp.tile(
                [NCORE, 512, w], BF16, name=f"hag_out{j}",
                tag=f"hag_out{j}", addr_space="Shared"))

        def cload(ap_in, shape, dt, tag, rearr=None, **kw):
            t = const.tile(list(shape), dt, name=tag, tag=tag)
            src = ap_in if rearr is None else ap_in.rearrange(rearr, **kw)
            nc.sync.dma_start(out=t[:], in_=src)
            return t

        # SBUF-resident constants.  [512, N] mats go in as [128, 4, N].
        eoT_s = cload(eoT, [128, 4, R], BF16, "eoT_s", "(kh kl) r -> kl kh r", kl=128)
        embsT_s = cload(embsT, [128, 4, RL], BF16, "embsT_s", "(kh kl) r -> kl kh r", kl=128)
        wq_s = cload(wq, [128, 4, 512], BF16, "wq_s", "(kh kl) m -> kl kh m", kl=128)
        wk_s = cload(wk, [128, 4, 512], BF16, "wk_s", "(kh kl) m -> kl kh m", kl=128)
        wv_s = cload(wv, [128, 4, 512], BF16, "wv_s", "(kh kl) m -> kl kh m", kl=128)
        wbeta_s = cload(wbeta, [128, 4, 512], BF16, "wbeta_s", "(kh kl) m -> kl kh m", kl=128)
        wih0_s = cload(wih0, [128, 4, 512], BF16, "wih0_s", "(kh kl) m -> kl kh m", kl=128)
        wic0_s = cload(wic0, [128, 4, 512], BF16, "wic0_s", "(kh kl) m -> kl kh m", kl=128)
        wtop_s = cload(wtop, [128, 4, 2048], BF16, "wtop_s", "(kh kl) m -> kl kh m", kl=128)
        wcat_s = cload(wcat, [128, 8, 2048], BF16, "wcat_s", "(kh kl) m -> kl kh m", kl=128)
        wfc_s = cload(wfc, [128, 4, VS], BF16, "wfc_s", "(kh kl) m -> kl kh m", kl=128)
        bq_s = cload(bq, [128, 4], F32, "bq_s")
        bk_s = cload(bk, [128, 4], F32, "bk_s")
        bbeta_s = cload(bbeta, [128, 4], F32, "bbeta_s")
        bih0_s = cload(bih0, [128, 4], F32, "bih0_s")
        bic0_s = cload(bic0, [128, 4], F32, "bic0_s")
        bcat_s = cload(bcat, [128, 16], F32, "bcat_s")
        bv_row_s = cload(bv_row, [1, 512], BF16, "bv_row_s")
        ones_row_s = cload(ones_row, [1, 128], BF16, "ones_row_s")
        ones_blk_s = cload(ones_blk, [128, NT, BL], F32, "ones_blk_s")
        onesT_blk_s = cload(onesT_blk, [8, NT, 128], F32, "onesT_blk_s")
        ones_col_s = cload(ones_col, [1, 128], F32, "ones_col_s")
        i8_s = cload(i8, [8, 8], F32, "i8_s")
        mask_sel_s = cload(mask_sel, [128, NS, 4, BL], F32, "mask_sel_s")
        mask_bd_s = cload(mask_bd, [128, NT, NS], F32, "mask_bd_s")
        maskrow_s = cload(maskrow, [1, ROWS], BF16, "maskrow_s")
        bfc_row_s = cload(bfc_row, [1, VS], BF16, "bfc_row_s")

        # persistent working tensors
        def st(shape, dt, tag):
            return state.tile(list(shape), dt, name=tag, tag=tag)

        att_kT_s = st([128, 4, R], BF16, "att_kT_s")     # [att_feat, (b,p)]
        att_v_s = st([128, NT, 512], BF16, "att_v_s")    # [(b,p), enc]
        e_all_s = st([128, NT, NS], F32, "e_all_s")      # exp(relu(s)) per step
        r_all_s = st([8, NS], F32, "r_all_s")            # 1/denominator per step
        E_all_s = st([128, 16, NS, BL], F32, "E_all_s")  # embs @ W_ih[:512] + b
        H_cT_s = st([128, 4, NS, BL], BF16, "H_cT_s")    # masked h_new, fmajor
        h_f32 = st([128, 4, BL], F32, "h_f32")
        c_f32 = st([128, 4, BL], F32, "c_f32")
        hT_bf = st([128, 4, BL], BF16, "hT_bf")
        mean_f = st([128, 4, BL], F32, "mean_f")
        mean_bf = st([128, 4, BL], BF16, "mean_bf")
        qT_bf = st([128, 4, BL], BF16, "qT_bf")
        beta_sig = st([128, 4, BL], F32, "beta_sig")
        beta_r = st([128, 4, BL], F32, "beta_r")
        xi_att = st([128, 4, BL], BF16, "xi_att")
        s_bd = st([128, NT], F32, "s_bd")
        sc_scr = st([128, NT, BL], F32, "sc_scr")
        e_blk = st([128, NT, BL], BF16, "e_blk")
        gpre = st([128, 16, BL], F32, "gpre")
        gact = st([128, 16, BL], F32, "gact")
        tmp1 = st([128, 4, BL], F32, "tmp1")
        tmp2 = st([128, 4, BL], F32, "tmp2")
        c_new = st([128, 4, BL], F32, "c_new")
        th_c = st([128, 4, BL], F32, "th_c")
        h_new = st([128, 4, BL], F32, "h_new")
        rT_sb = st([1, 8], F32, "rT_sb")
        alpha_st = st([128, NT, NS], F32, "alpha_st")
        H_T_s = st([128, 4, ROWS], BF16, "H_T_s")

        # zero the pad region of s_bd once; extraction never writes pads.
        nc.vector.memset(s_bd[:], 0.0)

        # ================= phase A: precompute =================
        with ExitStack() as pa:
            pc = pa.enter_context(tc.tile_pool(name="pc_ps", bufs=4, space="PSUM"))
            mm8 = pa.enter_context(tc.tile_pool(name="mm8a", bufs=2, space="PSUM"))

            # meanT over p: eoT_s viewed [128, 4, BL, P] reduce innermost
            nc.vector.tensor_reduce(
                out=mean_f[:],
                in_=eoT_s[:].rearrange("k e (b p) -> k e b p", p=P),
                axis=mybir.AxisListType.X,
                op=mybir.AluOpType.add,
            )
            nc.scalar.activation(
                out=mean_bf[:], in_=mean_f[:],
                func=mybir.ActivationFunctionType.Copy, scale=1.0 / P,
            )

            # h0 / c0  (feature-major [dec, b])
            for dst, w_s, b_s, zb in (
                (h_f32, wih0_s, bih0_s, zbias["b_init_h"]),
                (c_f32, wic0_s, bic0_s, zbias["b_init_c"]),
            ):
                ps = mm8.tile([128, 4, BL], F32, name="ps_h0", tag="mm8a")
                for mt in range(4):
                    for kt in range(4):
                        nc.tensor.matmul(
                            ps[:, mt, :],
                            w_s[:, kt, mt * 128:(mt + 1) * 128],
                            mean_bf[:, kt, :],
                            start=(kt == 0), stop=(kt == 3),
                        )
                if zb:
                    nc.scalar.activation(
                        out=dst[:], in_=ps[:],
                        func=mybir.ActivationFunctionType.Identity,
                    )
                else:
                    for mt in range(4):
                        nc.scalar.activation(
                            out=dst[:, mt, :], in_=ps[:, mt, :],
                            func=mybir.ActivationFunctionType.Identity,
                            bias=b_s[:, mt:mt + 1],
                        )
            nc.vector.tensor_copy(hT_bf[:], h_f32[:])

            # att_kT = (eo @ Wk + bk)^T : out [att_feat 4mt, (b,p)]
            cch = [(0, 512), (512, 512), (1024, 512), (1536, R - 1536)]
            for mt in range(4):
                for c0, csz in cch:
                    ps = pc.tile([128, 512], F32, name="ps_pc", tag="pc")
                    for kt in range(4):
                        nc.tensor.matmul(
                            ps[:, :csz],
                            wk_s[:, kt, mt * 128:(mt + 1) * 128],
                            eoT_s[:, kt, c0:c0 + csz],
                            start=(kt == 0), stop=(kt == 3),
                        )
                    if zbias["bk"]:
                        nc.scalar.activation(
                            out=att_kT_s[:, mt, c0:c0 + csz], in_=ps[:, :csz],
                            func=mybir.ActivationFunctionType.Identity,
                        )
                    else:
                        nc.scalar.activation(
                            out=att_kT_s[:, mt, c0:c0 + csz], in_=ps[:, :csz],
                            func=mybir.ActivationFunctionType.Identity,
                            bias=bk_s[:, mt:mt + 1],
                        )

            # att_v (b,p)-major: out [(b,p) 13mt, enc 512]
            for mt in range(NT):
                msz = min(128, R - mt * 128)
                ps = pc.tile([128, 512], F32, name="ps_pv", tag="pc")
                for kt in range(4):
                    nc.tensor.matmul(
                        ps[:msz, :],
                        eoT_s[:, kt, mt * 128:mt * 128 + msz],
                        wv_s[:, kt, :],
                        start=(kt == 0),
                        stop=(kt == 3 and zbias["bv"]),
                    )
                if not zbias["bv"]:
                    # + bv (rank-1; bv broadcast over rows)
                    nc.tensor.matmul(
                        ps[:msz, :],
                        ones_row_s[0:1, :msz],
                        bv_row_s[0:1, :],
                        start=False, stop=True,
                    )
                nc.vector.tensor_copy(att_v_s[:msz, mt, :], ps[:msz, :])

            # E_all = embs @ W_ih[:512] + (b_ih + b_hh)  (gate-feature-major)
            for mt in range(16):
                ps = pc.tile([128, RL], F32, name="ps_pe", tag="pc")
                for kt in range(4):
                    nc.tensor.matmul(
                        ps[:],
                        wtop_s[:, kt, mt * 128:(mt + 1) * 128],
                        embsT_s[:, kt, :],
                        start=(kt == 0), stop=(kt == 3),
                    )
                if zbias["b_ih_hh"]:
                    nc.scalar.activation(
                        out=E_all_s[:, mt, :, :].rearrange("p t b -> p (t b)"),
                        in_=ps[:],
                        func=mybir.ActivationFunctionType.Identity,
                    )
                else:
                    nc.scalar.activation(
                        out=E_all_s[:, mt, :, :].rearrange("p t b -> p (t b)"),
                        in_=ps[:],
                        func=mybir.ActivationFunctionType.Identity,
                        bias=bcat_s[:, mt:mt + 1],
                    )

        # ================= phase B: recurrence =================
        with ExitStack() as pb:
            mm8 = pb.enter_context(tc.tile_pool(name="mm8", bufs=1, space="PSUM"))
            scp = pb.enter_context(tc.tile_pool(name="scp", bufs=1, space="PSUM"))
            tny = pb.enter_context(tc.tile_pool(name="tny", bufs=3, space="PSUM"))

            for t in range(n_steps):
                mk = mask_sel_s[:, t, :, :]
                # ---- q = h @ Wq + bq   (feature-major out) ----
                q_ps = mm8.tile([128, 4, BL], F32, name="q_ps", tag="q_ps")
                for mt in range(4):
                    for kt in range(4):
                        nc.tensor.matmul(
                            q_ps[:, mt, :],
                            wq_s[:, kt, mt * 128:(mt + 1) * 128],
                            hT_bf[:, kt, :],
                            start=(kt == 0), stop=(kt == 3),
                        )
                if zbias["bq"]:
                    nc.scalar.activation(
                        out=qT_bf[:], in_=q_ps[:],
                        func=mybir.ActivationFunctionType.Identity,
                    )
                else:
                    for mt in range(4):
                        nc.scalar.activation(
                            out=qT_bf[:, mt, :], in_=q_ps[:, mt, :],
                            func=mybir.ActivationFunctionType.Identity,
                            bias=bq_s[:, mt:mt + 1],
                        )
                # ---- beta = sigmoid(h @ Wbeta + b) ----
                b_ps = mm8.tile([128, 4, BL], F32, name="b_ps", tag="b_ps")
                for mt in range(4):
                    for kt in range(4):
                        nc.tensor.matmul(
                            b_ps[:, mt, :],
                            wbeta_s[:, kt, mt * 128:(mt + 1) * 128],
                            hT_bf[:, kt, :],
                            start=(kt == 0), stop=(kt == 3),
                        )
                if zbias["b_beta"]:
                    nc.scalar.activation(
                        out=beta_sig[:], in_=b_ps[:],
                        func=mybir.ActivationFunctionType.Sigmoid,
                    )
                else:
                    for mt in range(4):
                        nc.scalar.activation(
                            out=beta_sig[:, mt, :], in_=b_ps[:, mt, :],
                            func=mybir.ActivationFunctionType.Sigmoid,
                            bias=bbeta_s[:, mt:mt + 1],
                        )

                # ---- scores (cross vs own q) into ONE psum tile ----
                sc_ps = scp.tile([128, NT, BL], F32, name="sc_ps", tag="sc_ps")
                for mt in range(NT):
                    msz = min(128, R - mt * 128)
                    for kt in range(4):
                        nc.tensor.matmul(
                            sc_ps[:msz, mt, :],
                            att_kT_s[:, kt, mt * 128:mt * 128 + msz],
                            qT_bf[:, kt, :],
                            start=(kt == 0), stop=(kt == 3),
                        )
                # diagonal extraction in two full-tile DVE ops; pad rows of
                # the last mtile hit zero blockdiag entries so they vanish.
                nc.vector.tensor_mul(sc_scr[:], sc_ps[:], ones_blk_s[:])
                nc.vector.tensor_reduce(
                    out=s_bd[:],
                    in_=sc_scr[:],
                    axis=mybir.AxisListType.X,
                    op=mybir.AluOpType.add,
                )

                # ---- exp(relu(s) * scale) ----
                nc.vector.tensor_scalar_max(s_bd[:], s_bd[:], 0.0)
                nc.scalar.activation(
                    out=e_all_s[:, :, t], in_=s_bd[:],
                    func=mybir.ActivationFunctionType.Exp, scale=SCALE,
                )

                # ---- denominators, reciprocal, partition-broadcast ----
                dn_ps = tny.tile([8, 1], F32, name="dn_ps", tag="tny")
                for kt in range(NT):
                    nc.tensor.matmul(
                        dn_ps[:],
                        ones_blk_s[:, kt, :],
                        e_all_s[:, kt, t:t + 1],
                        start=(kt == 0), stop=(kt == NT - 1),
                    )
                nc.vector.reciprocal(r_all_s[:, t:t + 1], dn_ps[:])
                rt_ps = tny.tile([1, 8], F32, name="rt_ps", tag="tny")
                nc.tensor.transpose(rt_ps[:], r_all_s[:, t:t + 1], i8_s[:])
                nc.scalar.copy(rT_sb[:], rt_ps[:])
                rbc_ps = tny.tile([128, 8], F32, name="rbc_ps", tag="tny")
                nc.tensor.matmul(
                    rbc_ps[:], ones_col_s[0:1, :], rT_sb[0:1, :],
                    start=True, stop=True,
                )
                for mt in range(4):
                    nc.vector.tensor_mul(
                        beta_r[:, mt, :], beta_sig[:, mt, :], rbc_ps[:]
                    )

                # ---- blockdiag unnormalized alpha ----
                for b in range(BL):
                    nc.vector.tensor_mul(
                        e_blk[:, :, b], ones_blk_s[:, :, b], e_all_s[:, :, t]
                    )

                # ---- att context (unnormalized), then * beta * (1/denom) ----
                a_ps = mm8.tile([128, 4, BL], F32, name="a_ps", tag="a_ps")
                for mt in range(4):
                    for kt in range(NT):
                        ksz = min(128, R - kt * 128)
                        nc.tensor.matmul(
                            a_ps[:, mt, :],
                            att_v_s[:ksz, kt, mt * 128:(mt + 1) * 128],
                            e_blk[:ksz, kt, :],
                            start=(kt == 0), stop=(kt == NT - 1),
                        )
                nc.vector.tensor_mul(xi_att[:], a_ps[:], beta_r[:])

                # ---- gates (gate-feature-major) into ONE psum tile ----
                g_ps = mm8.tile([128, 16, BL], F32, name="g_ps", tag="g_ps")
                for mt in range(16):
                    for kt in range(8):
                        rhs = xi_att[:, kt, :] if kt < 4 else hT_bf[:, kt - 4, :]
                        nc.tensor.matmul(
                            g_ps[:, mt, :],
                            wcat_s[:, kt, mt * 128:(mt + 1) * 128],
                            rhs,
                            start=(kt == 0), stop=(kt == 7),
                        )
                nc.vector.tensor_add(gpre[:], g_ps[:], E_all_s[:, :, t, :])
                nc.scalar.activation(
                    out=gact[:, 0:8, :], in_=gpre[:, 0:8, :],
                    func=mybir.ActivationFunctionType.Sigmoid,
                )
                nc.scalar.activation(
                    out=gact[:, 8:12, :], in_=gpre[:, 8:12, :],
                    func=mybir.ActivationFunctionType.Tanh,
                )
                nc.scalar.activation(
                    out=gact[:, 12:16, :], in_=gpre[:, 12:16, :],
                    func=mybir.ActivationFunctionType.Sigmoid,
                )

                # ---- LSTM cell (fp32 carries) ----
                nc.vector.tensor_mul(tmp1[:], gact[:, 4:8, :], c_f32[:])
                nc.vector.tensor_mul(tmp2[:], gact[:, 0:4, :], gact[:, 8:12, :])
                nc.vector.tensor_add(c_new[:], tmp1[:], tmp2[:])
                nc.scalar.activation(
                    out=th_c[:], in_=c_new[:],
                    func=mybir.ActivationFunctionType.Tanh,
                )
                nc.vector.tensor_mul(h_new[:], gact[:, 12:16, :], th_c[:])
                # masked prediction h -> H (bf16), masked state carry
                # (arithmetic blend: x += m * (x_new - x); mask is {0,1} f32)
                nc.vector.tensor_mul(H_cT_s[:, :, t, :], h_new[:], mk)
                nc.vector.tensor_sub(tmp1[:], h_new[:], h_f32[:])
                nc.vector.tensor_mul(tmp1[:], tmp1[:], mk)
                nc.vector.tensor_add(h_f32[:], h_f32[:], tmp1[:])
                nc.vector.tensor_sub(tmp2[:], c_new[:], c_f32[:])
                nc.vector.tensor_mul(tmp2[:], tmp2[:], mk)
                nc.vector.tensor_add(c_f32[:], c_f32[:], tmp2[:])
                nc.vector.tensor_copy(hT_bf[:], h_f32[:])

                if do_ag and (t + 1) in AG_CUTS:
                    j = AG_CUTS.index(t + 1) - 1
                    t0, t1 = AG_CUTS[j], AG_CUTS[j + 1]
                    nc.sync.dma_start(
                        out=hag_ins[j][:].rearrange(
                            "(eh el) (t b) -> el eh t b", el=128, t=t1 - t0
                        ),
                        in_=H_cT_s[:, :, t0:t1, :],
                    )
                    nc.gpsimd.collective_compute(
                        "AllGather",
                        mybir.AluOpType.bypass,
                        replica_groups=[list(range(NCORE))],
                        ins=[hag_ins[j][:].opt()],
                        outs=[hag_outs[j][:].opt()],
                    )
                    hro = hag_outs[j][:].rearrange(
                        "c (eh el) r -> c el eh r", el=128
                    )
                    for c in range(NCORE):
                        nc.sync.dma_start(
                            out=H_T_s[:, :,
                                      c * RL + t0 * BL:c * RL + t1 * BL],
                            in_=hro[c],
                        )

        # ================= alphas output =================
        with ExitStack() as pcx:
            r13 = pcx.enter_context(tc.tile_pool(name="r13", bufs=2, space="PSUM"))
            for mt in range(NT if do_alpha else 0):
                ps = r13.tile([128, NS], F32, name="ps_r13", tag="r13")
                nc.tensor.matmul(
                    ps[:], onesT_blk_s[:, mt, :], r_all_s[:], start=True, stop=True
                )
                nc.vector.tensor_mul(alpha_st[:, mt, :], e_all_s[:, mt, :], ps[:])
                nc.vector.tensor_mul(
                    alpha_st[:, mt, :], alpha_st[:, mt, :], mask_bd_s[:, mt, :]
                )
            if do_alpha:
                nc.sync.dma_start(out=alphas_o, in_=alpha_st[:])

        # (AllGather handled incrementally inside the step loop)

        # ================= phase C: fc =================
        with ExitStack() as pf:
            fcp = pf.enter_context(tc.tile_pool(name="fc_ps", bufs=6, space="PSUM"))
            stg = pf.enter_context(tc.tile_pool(name="fc_stage", bufs=8))
            for mt in range(NMT_FC if do_fc else 0):
                msz = min(128, ROWS - mt * 128)
                for (c0, csz) in chunks:
                    ps = fcp.tile([128, 512], F32, name="ps_fc", tag="fc")
                    for kt in range(4):
                        nc.tensor.matmul(
                            ps[:msz, :csz],
                            H_T_s[:, kt, mt * 128:mt * 128 + msz],
                            wfc_s[:, kt, c0:c0 + csz],
                            start=(kt == 0),
                            stop=(kt == 3 and not use_bfc),
                        )
                    if use_bfc:
                        nc.tensor.matmul(
                            ps[:msz, :csz],
                            maskrow_s[0:1, mt * 128:mt * 128 + msz],
                            bfc_row_s[0:1, c0:c0 + csz],
                            start=False, stop=True,
                        )
                    so = stg.tile([128, 512], F32, name="so_fc", tag="fcs")
                    if (mt + c0 // 512) % 2 == 0:
                        nc.vector.tensor_copy(so[:msz, :csz], ps[:msz, :csz])
                    else:
                        nc.scalar.copy(so[:msz, :csz], ps[:msz, :csz])
                    nc.sync.dma_start(
                        out=preds_o[mt * 128:mt * 128 + msz, c0:c0 + csz],
                        in_=so[:msz, :csz],
                    )

    fix_multiwait(nc)
    return nc


def _prep_inputs(inputs):
    """Host-side: sort, gather, transpose, cast. Returns (in_maps, host_outs)."""
    eo = np.asarray(inputs["encoder_out"], np.float32)
    caps_in = np.asarray(inputs["encoded_captions"])
    lens_in = np.asarray(inputs["caption_lens"])
    lens = lens_in[:, 0]
    sort_ind = np.argsort(-lens, kind="stable")
    lens_s = lens[sort_ind]
    eo_s = eo[sort_ind]                     # [B, P, ENC]
    caps_s = caps_in[sort_ind]              # [B, T]
    decode_len = lens_s - 1
    emb = np.asarray(inputs["emb"], np.float32)
    embs = emb[np.asarray(caps_s[:, :NS], np.int64)]  # [B, NS, EMB]

    w_ih = np.asarray(inputs["W_ih"], np.float32)
    w_hh = np.asarray(inputs["W_hh"], np.float32)
    b_fc = np.asarray(inputs["b_fc"], np.float32)
    use_bfc = bool(np.any(b_fc != 0.0))

    def fm(b):  # feature-major bias [128, nmt]
        b = np.asarray(b, np.float32)
        return np.ascontiguousarray(b.reshape(-1, 128).T)

    # blockdiag ones
    blk = np.zeros((NT * 128, BL), np.float32)
    rr = np.arange(R)
    blk[rr, rr // P] = 1.0
    ones_blk = np.ascontiguousarray(blk.reshape(NT, 128, BL).transpose(1, 0, 2))
    onesT_blk = np.ascontiguousarray(blk.reshape(NT, 128, BL).transpose(2, 0, 1))

    # masks: m[t, b] = t < decode_len[b]
    m_tb = (np.arange(NS)[:, None] < decode_len[None, :]).astype(np.float32)

    shared = dict(
        wq=np.asarray(inputs["Wq"], np.float32).astype(BF),
        wk=np.asarray(inputs["Wk"], np.float32).astype(BF),
        wv=np.asarray(inputs["Wv"], np.float32).astype(BF),
        wbeta=np.asarray(inputs["W_beta"], np.float32).astype(BF),
        wih0=np.asarray(inputs["W_init_h"], np.float32).astype(BF),
        wic0=np.asarray(inputs["W_init_c"], np.float32).astype(BF),
        wtop=np.ascontiguousarray(w_ih[:EMB]).astype(BF),
        wcat=np.ascontiguousarray(
            np.concatenate([w_ih[EMB:], w_hh], axis=0)
        ).astype(BF),
        bq=fm(inputs["bq"]),
        bk=fm(inputs["bk"]),
        bbeta=fm(inputs["b_beta"]),
        bih0=fm(inputs["b_init_h"]),
        bic0=fm(inputs["b_init_c"]),
        bcat=fm(np.asarray(inputs["b_ih"], np.float32)
                + np.asarray(inputs["b_hh"], np.float32)),
        bv_row=np.asarray(inputs["bv"], np.float32).reshape(1, ENC).astype(BF),
        ones_row=np.ones((1, 128), BF),
        ones_blk=ones_blk,
        onesT_blk=onesT_blk,
        ones_col=np.ones((1, 128), np.float32),
        i8=np.eye(8, dtype=np.float32),
    )

    # fc mask row over r_glob = c*152 + t*8 + b_local
    mrow = np.zeros((NCORE, NS, BL), np.float32)
    for c in range(NCORE):
        mrow[c] = m_tb[:, c * BL:(c + 1) * BL]
    shared["maskrow"] = mrow.reshape(1, ROWS).astype(BF)

    in_maps = []
    for c in range(NCORE):
        bsl = slice(c * BL, (c + 1) * BL)
        eoT = np.ascontiguousarray(
            eo_s[bsl].transpose(2, 0, 1).reshape(ENC, R)
        ).astype(BF)
        embsT = np.ascontiguousarray(
            embs[bsl].transpose(2, 1, 0).reshape(EMB, RL)
        ).astype(BF)
        m_c = m_tb[:, bsl]                             # [NS, BL]
        mask_sel = np.ascontiguousarray(
            np.broadcast_to(m_c[None, :, None, :], (128, NS, 4, BL))
        )
        mask_bd = np.zeros((NT * 128, NS), np.float32)
        mask_bd[rr] = m_c.T[rr // P]                   # [R rows, NS]
        mask_bd = np.ascontiguousarray(
            mask_bd.reshape(NT, 128, NS).transpose(1, 0, 2)
        )
        im = dict(shared)
        im.update(
            eoT=eoT,
            embsT=embsT,
            wfc=np.ascontiguousarray(
                np.asarray(inputs["W_fc"], np.float32)[:, c * VS:(c + 1) * VS]
            ).astype(BF),
            bfc_row=np.ascontiguousarray(b_fc[c * VS:(c + 1) * VS]).reshape(1, VS).astype(BF),
            mask_sel=mask_sel,
            mask_bd=mask_bd,
        )
        in_maps.append(im)

    def _z(name):
        return not bool(np.any(np.asarray(inputs[name], np.float32) != 0.0))

    zbias = dict(
        bq=_z("bq"), bk=_z("bk"), bv=_z("bv"), b_beta=_z("b_beta"),
        b_init_h=_z("b_init_h"), b_init_c=_z("b_init_c"),
        b_ih_hh=_z("b_ih") and _z("b_hh"),
    )
    host = dict(
        # match what the jax reference returns under default (x64-off) config
        caps=np.asarray(caps_s, np.int32),
        decode_len=np.asarray(decode_len, np.int32),
        sort_ind=sort_ind.astype(np.int32),
        use_bfc=use_bfc, zbias=zbias,
    )
    return in_maps, host


def kernel(**inputs):
    in_maps, host = _prep_inputs(inputs)
    nc = build_program(host["use_bfc"], host["zbias"])
    res = run_bass_kernel_spmd(
        nc, in_maps, core_ids=list(range(NCORE)), trace=False
    )
    LAST_RESULTS["res"] = res
    LAST_RESULTS["nc"] = nc

    predictions = np.empty((B, NS, V), np.float32)
    alphas = np.empty((B, NS, P), np.float32)
    for c in range(NCORE):
        out_c = res.results[c]
        pr = out_c["preds"].reshape(NCORE, NS, BL, VS)  # [c_batch, t, b_l, v]
        predictions[:, :, c * VS:(c + 1) * VS] = (
            pr.transpose(0, 2, 1, 3).reshape(B, NS, VS)
        )
        ar = out_c["alphas_raw"]                        # [128, NT, NS]
        flat = ar.transpose(1, 0, 2).reshape(NT * 128, NS)[:R]  # [(b,p), NS]
        alphas[c * BL:(c + 1) * BL] = (
            flat.reshape(BL, P, NS).transpose(0, 2, 1)
        )

    return (
        predictions,
        alphas,
        host["caps"],
        host["decode_len"],
        host["sort_ind"],
    )


if __name__ == "__main__":
    # smoke test with random inputs of the right shapes
    rng = np.random.default_rng(0)
    demo = dict(
        encoder_out=rng.standard_normal((B, P, ENC), np.float32),
        encoded_captions=rng.integers(0, V, (B, T)),
        caption_lens=rng.integers(2, T + 1, (B, 1)),
    )
    print("host prep only (no device):")
    in_maps, host = _prep_inputs(
        dict(
            demo,
            Wk=rng.standard_normal((ENC, ATT), np.float32) * 0.02,
            bk=np.zeros(ATT, np.float32),
            Wq=rng.standard_normal((DEC, ATT), np.float32) * 0.02,
            bq=np.zeros(ATT, np.float32),
            Wv=rng.standard_normal((ENC, ENC), np.float32) * 0.02,
            bv=np.zeros(ENC, np.float32),
            emb=rng.uniform(-0.1, 0.1, (V, EMB)).astype(np.float32),
            W_ih=rng.standard_normal((EMB + ENC, 4 * DEC), np.float32) * 0.02,
            b_ih=np.zeros(4 * DEC, np.float32),
            W_hh=rng.standard_normal((DEC, 4 * DEC), np.float32) * 0.02,
            b_hh=np.zeros(4 * DEC, np.float32),
            W_init_h=rng.standard_normal((ENC, DEC), np.float32) * 0.02,
            b_init_h=np.zeros(DEC, np.float32),
            W_init_c=rng.standard_normal((ENC, DEC), np.float32) * 0.02,
            b_init_c=np.zeros(DEC, np.float32),
            W_beta=rng.standard_normal((DEC, ENC), np.float32) * 0.02,
            b_beta=np.zeros(ENC, np.float32),
            W_fc=rng.standard_normal((DEC, V), np.float32) * 0.02,
            b_fc=np.zeros(V, np.float32),
        )
    )
    print("in_maps ready;", len(in_maps), "cores")
    nc = build_program(host["use_bfc"], host["zbias"])
    print("program built ok")
